# revision 10
# baseline (speedup 1.0000x reference)
"""FPS contact-map kernel for Trainium2 (8 NeuronCores, 2 batches/core).

Per FPS iteration (exact fp32 replica of the reference arithmetic):
  - squares (x-c)^2 per coord: three ACT Square passes (bias = -c per
    partition), planar [P, 3, W] mesh layout
  - s = (sqx + sqy) + sqz, D = min(D, s), row-max, row-argmax: five DVE
    passes (tensor_tensor_reduce / fused alternatives are illegal on TRN2)
  - tail: speculative 128-offset indirect DMA gathers every partition's
    candidate row right after MaxIndex; the global winner (max value,
    first-flat-index tie-break via partition_all_reduce in 2^23-n space)
    resolves concurrently; the winner's negated coords are one-hot masked
    and broadcast with a partition_all_reduce add (exact: 127 zeros + -c)
  - two batches per core run anti-phased; phase2b (post-DMA select) is
    emitted after both batches' main work so the DMA wait never blocks an
    in-order engine queue
GpSimd (Pool) is restricted to partition_all_reduce / memset / DMA
descriptor generation: any tensor_scalar there wedges real hardware.
"""

import math
import numpy as np

P = 128
N_FULL = 100000
B_FULL = 16
NPOINT_FULL = 1024
NCORES = 8
BPC = 2

C23 = float(2 ** 23)

_BUILD_CACHE = {}


def _build(N, NPOINT, UNROLL, WZA=261, ADD_DVE=200, debug=False):
    """WZA: z-coord square columns on ACT (rest: DVE sub + Pool mult).
    ADD_DVE: add columns on DVE (rest on Pool)."""
    import concourse.bass as bass
    import concourse.bacc as bacc
    import concourse.mybir as mybir
    from concourse.tile import TileContext
    from concourse.bass_isa import ReduceOp

    W = math.ceil(N / P)
    FP32 = mybir.dt.float32
    I32 = mybir.dt.int32
    U32 = mybir.dt.uint32
    Alu = mybir.AluOpType
    Act = mybir.ActivationFunctionType
    X = mybir.AxisListType.X
    NG = NPOINT // P
    assert NPOINT % P == 0
    WZ_ = min(WZA, W)
    AD_ = min(ADD_DVE, W)

    nc = bacc.Bacc("TRN2", target_bir_lowering=False, debug=False)

    mesh_sb_in = nc.dram_tensor("mesh_sb", [BPC, P, 3 * W], FP32, kind="ExternalInput")
    meshflat_in = nc.dram_tensor("meshflat", [BPC * P * W, 3], FP32, kind="ExternalInput")
    cmflat_in = nc.dram_tensor("cmflat", [BPC * P * W, 1], FP32, kind="ExternalInput")
    d0_in = nc.dram_tensor("d0", [BPC, P, W], FP32, kind="ExternalInput")
    negc0_in = nc.dram_tensor("negc0", [BPC, P, 3], FP32, kind="ExternalInput")
    pwfm_in = nc.dram_tensor("pwfm", [P, 1], FP32, kind="ExternalInput")
    pwfbn_in = nc.dram_tensor("pwfbn", [BPC, P, 1], FP32, kind="ExternalInput")
    pwf_in = nc.dram_tensor("pwf", [P, 1], FP32, kind="ExternalInput")
    centinit_in = nc.dram_tensor("centinit", [1, BPC], FP32, kind="ExternalInput")

    out_t = nc.dram_tensor("out", [BPC, NPOINT, 4], FP32, kind="ExternalOutput")
    if debug:
        cent_out = nc.dram_tensor("cent", [BPC, NPOINT], FP32, kind="ExternalOutput")

    with TileContext(nc) as tc:
        with tc.tile_pool(name="persist", bufs=1) as cp, \
             tc.tile_pool(name="work", bufs=3) as wp, \
             tc.tile_pool(name="psum", bufs=1, space="PSUM") as pp:

            from concourse.masks import make_identity
            ident = cp.tile([P, P], FP32, name="ident", tag="ident")
            make_identity(nc, ident[:])
            pwfm = cp.tile([P, 1], FP32, name="pwfm", tag="pwfm")
            nc.sync.dma_start(out=pwfm[:], in_=pwfm_in[:])
            pwf = cp.tile([P, 1], FP32, name="pwf", tag="pwf")
            nc.sync.dma_start(out=pwf[:], in_=pwf_in[:])
            negones = cp.tile([2, P], FP32, name="negones", tag="negones")
            nc.gpsimd.memset(negones[:], -0.5)
            HB = 512  # psum bank size in fp32

            msb, D, pm8, pidx, negc, cent = [], [], [], [], [], []
            sqx, sqy_sq, s12, sv = [], [], [], []
            s_ps, crow, negc_ps = [], [], []
            npfi, cand128, pwfbn = [], [], []
            npfm, mskd, wmax, ni32, km, candn, masked, gbc = [], [], [], [], [], [], [], []
            boffP = []
            for b in range(BPC):
                msb.append(cp.tile([P, 3 * W], FP32, name=f"msb{b}", tag=f"msb{b}"))
                D.append(cp.tile([P, W], FP32, name=f"D{b}", tag=f"D{b}"))
                pm8.append([cp.tile([P, 8], FP32, name=f"pm8{b}_{q}", tag=f"pm8{b}_{q}")
                            for q in range(2)])
                pidx.append([cp.tile([P, 8], U32, name=f"pidx{b}_{q}", tag=f"pidx{b}_{q}")
                             for q in range(2)])
                negc.append(cp.tile([P, 3], FP32, name=f"negc{b}", tag=f"negc{b}"))
                cent.append(cp.tile([1, NPOINT], FP32, name=f"cent{b}", tag=f"cent{b}"))
                sqx.append(cp.tile([P, W], FP32, name=f"sqx{b}", tag=f"sqx{b}"))
                sqy_sq.append(cp.tile([P, 2 * W], FP32, name=f"sqy{b}", tag=f"sqy{b}"))
                sv.append(cp.tile([P, W], FP32, name=f"sv{b}", tag=f"sv{b}"))
                npfi.append(cp.tile([P, 1], I32, name=f"npfi{b}", tag=f"npfi{b}"))
                cand128.append(cp.tile([P, 3], FP32, name=f"cand128{b}", tag=f"cand128{b}"))
                pwfbn.append(cp.tile([P, 1], FP32, name=f"pwfbn{b}", tag=f"pwfbn{b}"))
                nc.sync.dma_start(out=pwfbn[b][:], in_=pwfbn_in[b])
                npfm.append(cp.tile([P, 1], FP32, name=f"npfm{b}", tag=f"npfm{b}"))
                mskd.append(cp.tile([P, 1], FP32, name=f"mskd{b}", tag=f"mskd{b}"))
                wmax.append(cp.tile([P, 1], FP32, name=f"wmax{b}", tag=f"wmax{b}"))
                ni32.append(cp.tile([2, 1], I32, name=f"ni32{b}", tag=f"ni32{b}"))
                km.append(cp.tile([P, 1], FP32, name=f"km{b}", tag=f"km{b}"))
                candn.append(cp.tile([P, 3], FP32, name=f"candn{b}", tag=f"candn{b}"))
                masked.append(cp.tile([P, 3], FP32, name=f"masked{b}", tag=f"masked{b}"))
                gbc.append(cp.tile([P, 1], FP32, name=f"gbc{b}", tag=f"gbc{b}"))
                boffP.append(cp.tile([P, 1], FP32, name=f"boffP{b}", tag=f"boffP{b}"))
                nc.gpsimd.memset(boffP[b][:], float(b * P * W))

                nc.sync.dma_start(out=msb[b][:], in_=mesh_sb_in[b])
                nc.sync.dma_start(out=D[b][:], in_=d0_in[b])
                nc.sync.dma_start(out=negc[b][:], in_=negc0_in[b])
                nc.sync.dma_start(out=cent[b][0:1, 0:1], in_=centinit_in[0:1, b:b + 1])
                nc.gpsimd.memset(pm8[b][0][:], -1e30)
                nc.gpsimd.memset(pm8[b][1][:], -1e30)
                nc.gpsimd.memset(sqy_sq[b][:], 0.0)

            # planar views: plane(b, c) = msb[b][:, c*W:(c+1)*W]
            def plane(b, c):
                return msb[b][:, c * W:(c + 1) * W]

            tc.strict_bb_all_engine_barrier()

            def phase0_xy(b):
                sqy = sqy_sq[b][:, 0:W]
                nc.scalar.activation(out=sqx[b][:], in_=plane(b, 0),
                                     func=Act.Square, bias=negc[b][:, 0:1], scale=1.0)
                nc.scalar.activation(out=sqy[:], in_=plane(b, 1),
                                     func=Act.Square, bias=negc[b][:, 1:2], scale=1.0)

            def phase0_z(b):
                sqz = sqy_sq[b][:, W:2 * W]
                nc.scalar.activation(out=sqz[:], in_=plane(b, 2),
                                     func=Act.Square, bias=negc[b][:, 2:3], scale=1.0)

            def phase1(b, q):
                sqy = sqy_sq[b][:, 0:W]
                sqz = sqy_sq[b][:, W:2 * W]
                nc.vector.tensor_tensor(out=sv[b][:], in0=sqx[b][:], in1=sqy[:],
                                        op=Alu.add)
                nc.vector.tensor_tensor(out=sv[b][:], in0=sv[b][:], in1=sqz[:],
                                        op=Alu.add)
                nc.vector.tensor_tensor(out=D[b][:], in0=D[b][:], in1=sv[b][:],
                                        op=Alu.min)
                nc.vector.reduce_max(out=pm8[b][q][:, 0:1], in_=D[b][:], axis=X)
                nc.vector.max_index(out=pidx[b][q][:], in_max=pm8[b][q][:],
                                    in_values=D[b][:])

            def phase2a(b, q):
                # pre-DMA: speculative gather of all 128 candidates + winner
                # resolution (runs concurrently with the DMA)
                nc.vector.tensor_scalar(out=npfi[b][:], in0=pidx[b][q][:, 0:1],
                                        scalar1=1.0, scalar2=pwfbn[b][:],
                                        op0=Alu.mult, op1=Alu.add)
                nc.gpsimd.partition_all_reduce(gbc[b][:], pm8[b][q][:, 0:1], P,
                                               ReduceOp.max)
                nc.gpsimd.indirect_dma_start(
                    out=cand128[b][:], out_offset=None, in_=meshflat_in[:],
                    in_offset=bass.IndirectOffsetOnAxis(ap=npfi[b][:, 0:1], axis=0))
                nc.vector.tensor_scalar(out=npfm[b][:], in0=pidx[b][q][:, 0:1],
                                        scalar1=-1.0, scalar2=pwfm[:],
                                        op0=Alu.mult, op1=Alu.add)
                nc.vector.scalar_tensor_tensor(
                    out=mskd[b][:], in0=pm8[b][q][:, 0:1], scalar=gbc[b][:],
                    in1=npfm[b][:], op0=Alu.is_equal, op1=Alu.mult)
                nc.gpsimd.partition_all_reduce(wmax[b][:], mskd[b][:], P,
                                               ReduceOp.max)
                nc.vector.tensor_scalar(out=km[b][:], in0=mskd[b][:],
                                        scalar1=wmax[b][:], scalar2=None,
                                        op0=Alu.is_equal)

            def phase2b(b):
                # post-DMA: one-hot select + broadcast (emitted after both
                # batches' main work so the DMA wait never blocks a queue)
                nc.vector.tensor_scalar(out=masked[b][:], in0=cand128[b][:],
                                        scalar1=km[b][:], scalar2=-1.0,
                                        op0=Alu.mult, op1=Alu.mult)
                nc.gpsimd.partition_all_reduce(negc[b][:], masked[b][:], P,
                                               ReduceOp.add)

            def body_iter(k_aps, q):
                for b in range(BPC):
                    phase0_xy(b)
                    phase0_z(b)
                for b in range(BPC):
                    phase1(b, q)
                    phase2a(b, q)
                for b in range(BPC):
                    phase2b(b)
                for b in range(BPC):
                    nc.vector.tensor_scalar(out=cent[b][0:1, k_aps[b]],
                                            in0=wmax[b][0:1, 0:1], scalar1=-1.0,
                                            scalar2=C23, op0=Alu.mult, op1=Alu.add)

            n_iters = NPOINT - 1
            n_loop = (n_iters // UNROLL) * UNROLL
            if n_loop > 0:
                with tc.For_i(1, 1 + n_loop, UNROLL) as i:
                    for u in range(UNROLL):
                        body_iter([bass.ds(i + u, 1)] * BPC, u % 2)
            for k in range(1 + n_loop, NPOINT):
                body_iter([slice(k, k + 1)] * BPC, k % 2)

            # ---- final stage: s_obj, gather, normalize, emit ----
            from concourse.bass_isa import ReduceOp as RO
            for b in range(BPC):
                planes3 = msb[b][:].rearrange("p (c w) -> p c w", c=3)
                colsum = wp.tile([P, 3], FP32, name="colsum", tag="colsum")
                nc.vector.reduce_sum(out=colsum[:], in_=planes3, axis=X)
                sums_bc = wp.tile([P, 3], FP32, name="sumsbc", tag="sumsbc")
                nc.gpsimd.partition_all_reduce(sums_bc[:], colsum[:], P, RO.add)
                negmean = wp.tile([P, 3], FP32, name="negmean", tag="negmean")
                nc.vector.tensor_scalar(out=negmean[:], in0=sums_bc[:],
                                        scalar1=-1.0 / N, scalar2=None, op0=Alu.mult)
                sq = [wp.tile([P, W], FP32, name=f"fsq{c}", tag=f"fsq{c}") for c in range(3)]
                for c in range(3):
                    nc.scalar.activation(
                        out=sq[c][:], in_=plane(b, c),
                        func=Act.Square, bias=negmean[:, c:c + 1], scale=1.0)
                s = wp.tile([P, W], FP32, name="fs", tag="fs")
                nc.vector.tensor_tensor(out=s[:], in0=sq[0][:], in1=sq[1][:], op=Alu.add)
                nc.vector.tensor_tensor(out=s[:], in0=s[:], in1=sq[2][:], op=Alu.add)
                mcol = wp.tile([P, 1], FP32, name="mcol", tag="mcol")
                nc.vector.reduce_max(out=mcol[:], in_=s[:], axis=X)
                mbc = wp.tile([P, 1], FP32, name="mbc", tag="mbc")
                nc.gpsimd.partition_all_reduce(mbc[:], mcol[:], P, RO.max)
                scol = wp.tile([P, 1], FP32, name="scol", tag="scol")
                nc.scalar.activation(out=scol[:], in_=mbc[:], func=Act.Sqrt)
                y0 = wp.tile([P, 1], FP32, name="y0", tag="y0")
                nc.vector.reciprocal(out=y0[:], in_=scol[:])
                t0 = wp.tile([P, 1], FP32, name="t0", tag="t0")
                nc.vector.tensor_tensor(out=t0[:], in0=mbc[:], in1=y0[:], op=Alu.mult)
                nc.vector.tensor_tensor(out=t0[:], in0=t0[:], in1=y0[:], op=Alu.mult)
                nc.vector.tensor_scalar(out=t0[:], in0=t0[:], scalar1=-0.5,
                                        scalar2=1.5, op0=Alu.mult, op1=Alu.add)
                y1 = wp.tile([P, 1], FP32, name="y1", tag="y1")
                nc.vector.tensor_tensor(out=y1[:], in0=y0[:], in1=t0[:], op=Alu.mult)

                centT = pp.tile([P, NG], FP32, name="centT", tag="centT")
                for j in range(NG):
                    nc.tensor.transpose(out=centT[:, j:j + 1],
                                        in_=cent[b][0:1, j * P:(j + 1) * P],
                                        identity=ident[0:1, 0:1])
                offsg = wp.tile([P, NG], I32, name="offsg", tag="offsg")
                nc.scalar.activation(out=offsg[:], in_=centT[:],
                                     func=Act.Identity, bias=boffP[b][:], scale=1.0)
                obuf = wp.tile([P, NG * 4], FP32, name="obuf", tag="obuf")
                for j in range(NG):
                    pcj = wp.tile([P, 3], FP32, name="pcj", tag="pcj")
                    nc.gpsimd.indirect_dma_start(
                        out=pcj[:], out_offset=None, in_=meshflat_in[:],
                        in_offset=bass.IndirectOffsetOnAxis(ap=offsg[:, j:j + 1], axis=0))
                    nc.gpsimd.indirect_dma_start(
                        out=obuf[:, 4 * j:4 * j + 1], out_offset=None, in_=cmflat_in[:],
                        in_offset=bass.IndirectOffsetOnAxis(ap=offsg[:, j:j + 1], axis=0))
                    nc.vector.tensor_scalar(out=obuf[:, 4 * j + 1:4 * j + 4],
                                            in0=pcj[:], scalar1=y1[:], scalar2=None,
                                            op0=Alu.mult)
                out_ap = out_t[b].rearrange("(j p) f -> p j f", p=P)
                nc.sync.dma_start(out=out_ap,
                                  in_=obuf[:].rearrange("p (j f) -> p j f", f=4))
                if debug:
                    nc.sync.dma_start(out=cent_out[b].rearrange("(o n) -> o n", o=1),
                                      in_=cent[b][:])

    nc.finalize()
    return nc, W


def _prep_core_inputs(mesh, contact, init, N, W, core):
    """Per-core input map for batches [2c, 2c+1]; planar mesh layout."""
    bsel = [BPC * core + i for i in range(BPC)]
    mesh_sb = np.zeros((BPC, P, 3 * W), np.float32)
    d0 = np.full((BPC, P * W), -1.0, np.float32)
    negc0 = np.zeros((BPC, P, 3), np.float32)
    centinit = np.zeros((1, BPC), np.float32)
    for i, b in enumerate(bsel):
        buf = np.zeros((P * W, 3), np.float32)
        buf[:N] = mesh[b]
        # [P, W, 3] -> [P, 3, W] planar
        mesh_sb[i] = buf.reshape(P, W, 3).transpose(0, 2, 1).reshape(P, 3 * W)
        d0[i, :N] = 1e10
        init_idx = int(init[b])
        negc0[i, :, :] = -mesh[b, init_idx][None, :]
        centinit[0, i] = float(init_idx)
    d0 = d0.reshape(BPC, P, W)
    PW = P * W
    meshflat = np.zeros((BPC * PW, 3), np.float32)
    cmflat = np.zeros((BPC * PW, 1), np.float32)
    for i, b in enumerate(bsel):
        meshflat[i * PW:i * PW + N] = mesh[b]
        cmflat[i * PW:i * PW + N] = contact[b]
    pwfm = (C23 - np.arange(P, dtype=np.float32) * W).reshape(P, 1)
    pwf = (np.arange(P, dtype=np.float32) * W).reshape(P, 1)
    pwfbn = np.stack([(np.arange(P, dtype=np.float32) * W + b * PW).reshape(P, 1)
                      for b in range(BPC)])
    return {
        "mesh_sb": mesh_sb, "meshflat": meshflat, "cmflat": cmflat,
        "d0": d0, "negc0": negc0, "pwfm": pwfm, "pwf": pwf, "pwfbn": pwfbn,
        "centinit": centinit,
    }


def _get_built(N=N_FULL, NPOINT=NPOINT_FULL, UNROLL=31, WZA=261, ADD_DVE=250,
               debug=False):
    key = (N, NPOINT, UNROLL, WZA, ADD_DVE, debug)
    if key not in _BUILD_CACHE:
        _BUILD_CACHE[key] = _build(N, NPOINT, UNROLL, WZA, ADD_DVE, debug)
    return _BUILD_CACHE[key]


def kernel(mesh, contact_map, init_farthest, npoint):
    from concourse.bass_utils import run_bass_kernel_spmd

    mesh = np.asarray(mesh, np.float32)
    contact_map = np.asarray(contact_map, np.float32)
    init_farthest = np.asarray(init_farthest, np.int32)
    assert int(npoint) == NPOINT_FULL and mesh.shape == (B_FULL, N_FULL, 3)

    nc, W = _get_built()
    in_maps = [
        _prep_core_inputs(mesh, contact_map, init_farthest, N_FULL, W, c)
        for c in range(NCORES)
    ]
    res = run_bass_kernel_spmd(nc, in_maps, list(range(NCORES)))
    out = np.concatenate([res.results[c]["out"] for c in range(NCORES)], axis=0)
    return out.astype(np.float32)


# revision 13
# speedup vs baseline: 1.1239x; 1.1239x over previous
"""FPS contact-map kernel for Trainium2 (8 NeuronCores, 2 batches/core).

Per FPS iteration (exact fp32 replica of the reference arithmetic):
  - squares (x-c)^2 per coord: three ACT Square passes (bias = -c per
    partition), planar [P, 3, W] mesh layout
  - s = (sqx + sqy) + sqz, D = min(D, s), row-max, row-argmax: five DVE
    passes (tensor_tensor_reduce / fused alternatives are illegal on TRN2)
  - tail: speculative 128-offset indirect DMA gathers every partition's
    candidate row right after MaxIndex; the global winner (max value,
    first-flat-index tie-break via partition_all_reduce in 2^23-n space)
    resolves concurrently; the winner's negated coords are one-hot masked
    and broadcast with a partition_all_reduce add (exact: 127 zeros + -c)
  - two batches per core run anti-phased; phase2b (post-DMA select) is
    emitted after both batches' main work so the DMA wait never blocks an
    in-order engine queue
GpSimd (Pool) is restricted to partition_all_reduce / memset / DMA
descriptor generation: any tensor_scalar there wedges real hardware.

Known headroom (next session): the two batches settle into LOCKSTEP
(one shared 2.4us DVE gap per 11.45us period while both DMA windows
coincide). A true anti-phase equilibrium (period ~9.6us = per-batch
chain: 1.8us squares + 4.4us DVE block + 3.4us gather) exists but a forced
stagger via a dummy ACT dependency on the other batch's row-max was
tested and changed nothing (sim-identical schedule): the binding
constraint is NOT the other batch's square start. Next step: dump both
batches' per-op timestamps for one period and find which queue edge
actually pins the 2.4us gap before attempting another fix.
"""

import math
import numpy as np

P = 128
N_FULL = 100000
B_FULL = 16
NPOINT_FULL = 1024
NCORES = 8
BPC = 2

C23 = float(2 ** 23)

_BUILD_CACHE = {}


def _build(N, NPOINT, UNROLL, WZA=261, ADD_DVE=200, debug=False):
    """WZA: z-coord square columns on ACT (rest: DVE sub + Pool mult).
    ADD_DVE: add columns on DVE (rest on Pool)."""
    import concourse.bass as bass
    import concourse.bacc as bacc
    import concourse.mybir as mybir
    from concourse.tile import TileContext
    from concourse.bass_isa import ReduceOp

    W = math.ceil(N / P)
    FP32 = mybir.dt.float32
    I32 = mybir.dt.int32
    U32 = mybir.dt.uint32
    Alu = mybir.AluOpType
    Act = mybir.ActivationFunctionType
    X = mybir.AxisListType.X
    NG = NPOINT // P
    assert NPOINT % P == 0
    WZ_ = min(WZA, W)
    AD_ = min(ADD_DVE, W)

    nc = bacc.Bacc("TRN2", target_bir_lowering=False, debug=False)

    mesh_sb_in = nc.dram_tensor("mesh_sb", [BPC, P, 3 * W], FP32, kind="ExternalInput")
    meshflat_in = nc.dram_tensor("meshflat", [BPC * P * W, 3], FP32, kind="ExternalInput")
    cmflat_in = nc.dram_tensor("cmflat", [BPC * P * W, 1], FP32, kind="ExternalInput")
    d0_in = nc.dram_tensor("d0", [BPC, P, W], FP32, kind="ExternalInput")
    negc0_in = nc.dram_tensor("negc0", [BPC, P, 3], FP32, kind="ExternalInput")
    pwfm_in = nc.dram_tensor("pwfm", [P, 1], FP32, kind="ExternalInput")
    pwfbn_in = nc.dram_tensor("pwfbn", [BPC, P, 1], FP32, kind="ExternalInput")
    pwf_in = nc.dram_tensor("pwf", [P, 1], FP32, kind="ExternalInput")
    centinit_in = nc.dram_tensor("centinit", [1, BPC], FP32, kind="ExternalInput")

    out_t = nc.dram_tensor("out", [BPC, NPOINT, 4], FP32, kind="ExternalOutput")
    if debug:
        cent_out = nc.dram_tensor("cent", [BPC, NPOINT], FP32, kind="ExternalOutput")

    with TileContext(nc) as tc:
        with tc.tile_pool(name="persist", bufs=1) as cp, \
             tc.tile_pool(name="work", bufs=3) as wp, \
             tc.tile_pool(name="psum", bufs=1, space="PSUM") as pp:

            from concourse.masks import make_identity
            ident = cp.tile([P, P], FP32, name="ident", tag="ident")
            make_identity(nc, ident[:])
            pwfm = cp.tile([P, 1], FP32, name="pwfm", tag="pwfm")
            nc.sync.dma_start(out=pwfm[:], in_=pwfm_in[:])
            pwf = cp.tile([P, 1], FP32, name="pwf", tag="pwf")
            nc.sync.dma_start(out=pwf[:], in_=pwf_in[:])
            negones = cp.tile([2, P], FP32, name="negones", tag="negones")
            nc.gpsimd.memset(negones[:], -0.5)
            HB = 512  # psum bank size in fp32

            msb, D, pm8, pidx, negc, cent = [], [], [], [], [], []
            sqx, sqy_sq, s12, sv = [], [], [], []
            s_ps, crow, negc_ps = [], [], []
            npfi, cand128, pwfbn = [], [], []
            npfm, mskd, wmax, ni32, km, candn, masked, gbc = [], [], [], [], [], [], [], []
            boffP = []
            for b in range(BPC):
                msb.append(cp.tile([P, 3 * W], FP32, name=f"msb{b}", tag=f"msb{b}"))
                D.append(cp.tile([P, W], FP32, name=f"D{b}", tag=f"D{b}"))
                pm8.append([cp.tile([P, 8], FP32, name=f"pm8{b}_{q}", tag=f"pm8{b}_{q}")
                            for q in range(2)])
                pidx.append([cp.tile([P, 8], U32, name=f"pidx{b}_{q}", tag=f"pidx{b}_{q}")
                             for q in range(2)])
                negc.append(cp.tile([P, 3], FP32, name=f"negc{b}", tag=f"negc{b}"))
                cent.append(cp.tile([1, NPOINT], FP32, name=f"cent{b}", tag=f"cent{b}"))
                sqx.append(cp.tile([P, W], FP32, name=f"sqx{b}", tag=f"sqx{b}"))
                sqy_sq.append(cp.tile([P, 2 * W], FP32, name=f"sqy{b}", tag=f"sqy{b}"))
                sv.append(cp.tile([P, W], FP32, name=f"sv{b}", tag=f"sv{b}"))
                npfi.append(cp.tile([P, 1], I32, name=f"npfi{b}", tag=f"npfi{b}"))
                cand128.append(cp.tile([P, 3], FP32, name=f"cand128{b}", tag=f"cand128{b}"))
                pwfbn.append(cp.tile([P, 1], FP32, name=f"pwfbn{b}", tag=f"pwfbn{b}"))
                nc.sync.dma_start(out=pwfbn[b][:], in_=pwfbn_in[b])
                npfm.append(cp.tile([P, 1], FP32, name=f"npfm{b}", tag=f"npfm{b}"))
                mskd.append(cp.tile([P, 1], FP32, name=f"mskd{b}", tag=f"mskd{b}"))
                wmax.append(cp.tile([P, 1], FP32, name=f"wmax{b}", tag=f"wmax{b}"))
                ni32.append(cp.tile([2, 1], I32, name=f"ni32{b}", tag=f"ni32{b}"))
                km.append(cp.tile([P, 1], FP32, name=f"km{b}", tag=f"km{b}"))
                candn.append(cp.tile([P, 3], FP32, name=f"candn{b}", tag=f"candn{b}"))
                masked.append(cp.tile([P, 3], FP32, name=f"masked{b}", tag=f"masked{b}"))
                gbc.append(cp.tile([P, 1], FP32, name=f"gbc{b}", tag=f"gbc{b}"))
                boffP.append(cp.tile([P, 1], FP32, name=f"boffP{b}", tag=f"boffP{b}"))
                nc.gpsimd.memset(boffP[b][:], float(b * P * W))

                nc.sync.dma_start(out=msb[b][:], in_=mesh_sb_in[b])
                nc.sync.dma_start(out=D[b][:], in_=d0_in[b])
                nc.sync.dma_start(out=negc[b][:], in_=negc0_in[b])
                nc.sync.dma_start(out=cent[b][0:1, 0:1], in_=centinit_in[0:1, b:b + 1])
                nc.gpsimd.memset(pm8[b][0][:], -1e30)
                nc.gpsimd.memset(pm8[b][1][:], -1e30)
                nc.gpsimd.memset(pidx[b][0][:], 0)
                nc.gpsimd.memset(pidx[b][1][:], 0)
                nc.gpsimd.memset(sqy_sq[b][:], 0.0)

            # planar views: plane(b, c) = msb[b][:, c*W:(c+1)*W]
            def plane(b, c):
                return msb[b][:, c * W:(c + 1) * W]

            tc.strict_bb_all_engine_barrier()

            def phase0_xy(b):
                sqy = sqy_sq[b][:, 0:W]
                nc.scalar.activation(out=sqx[b][:], in_=plane(b, 0),
                                     func=Act.Square, bias=negc[b][:, 0:1], scale=1.0)
                nc.scalar.activation(out=sqy[:], in_=plane(b, 1),
                                     func=Act.Square, bias=negc[b][:, 1:2], scale=1.0)

            def phase0_z(b):
                sqz = sqy_sq[b][:, W:2 * W]
                nc.scalar.activation(out=sqz[:], in_=plane(b, 2),
                                     func=Act.Square, bias=negc[b][:, 2:3], scale=1.0)

            def phase1(b, q):
                sqy = sqy_sq[b][:, 0:W]
                sqz = sqy_sq[b][:, W:2 * W]
                nc.vector.tensor_tensor(out=sv[b][:], in0=sqx[b][:], in1=sqy[:],
                                        op=Alu.add)
                nc.vector.tensor_tensor(out=sv[b][:], in0=sv[b][:], in1=sqz[:],
                                        op=Alu.add)
                nc.vector.tensor_tensor(out=D[b][:], in0=D[b][:], in1=sv[b][:],
                                        op=Alu.min)
                nc.vector.reduce_max(out=pm8[b][q][:, 0:1], in_=D[b][:], axis=X)
                nc.vector.max_index(out=pidx[b][q][:], in_max=pm8[b][q][:],
                                    in_values=D[b][:])

            def phase2a(b, q):
                # pre-DMA: speculative gather of all 128 candidates + winner
                # resolution (runs concurrently with the DMA)
                nc.vector.tensor_scalar(out=npfi[b][:], in0=pidx[b][q][:, 0:1],
                                        scalar1=1.0, scalar2=pwfbn[b][:],
                                        op0=Alu.mult, op1=Alu.add)
                nc.gpsimd.partition_all_reduce(gbc[b][:], pm8[b][q][:, 0:1], P,
                                               ReduceOp.max)
                nc.gpsimd.indirect_dma_start(
                    out=cand128[b][:], out_offset=None, in_=meshflat_in[:],
                    in_offset=bass.IndirectOffsetOnAxis(ap=npfi[b][:, 0:1], axis=0))
                nc.vector.tensor_scalar(out=npfm[b][:], in0=pidx[b][q][:, 0:1],
                                        scalar1=-1.0, scalar2=pwfm[:],
                                        op0=Alu.mult, op1=Alu.add)
                nc.vector.scalar_tensor_tensor(
                    out=mskd[b][:], in0=pm8[b][q][:, 0:1], scalar=gbc[b][:],
                    in1=npfm[b][:], op0=Alu.is_equal, op1=Alu.mult)
                nc.gpsimd.partition_all_reduce(wmax[b][:], mskd[b][:], P,
                                               ReduceOp.max)
                nc.vector.tensor_scalar(out=km[b][:], in0=mskd[b][:],
                                        scalar1=wmax[b][:], scalar2=None,
                                        op0=Alu.is_equal)

            def phase2b(b):
                # post-DMA: one-hot select + broadcast (emitted after both
                # batches' main work so the DMA wait never blocks a queue)
                nc.vector.tensor_scalar(out=masked[b][:], in0=cand128[b][:],
                                        scalar1=km[b][:], scalar2=-1.0,
                                        op0=Alu.mult, op1=Alu.mult)
                nc.gpsimd.partition_all_reduce(negc[b][:], masked[b][:], P,
                                               ReduceOp.add)

            def fence(b, other_pidx):
                # value-preserving touch: sqx[b][0,0] = 0*other_pidx + sqx[b][0,0].
                # Forces batch b's first add to wait for the other batch's
                # MaxIndex, forbidding the scheduler from interleaving b's adds
                # into the other batch's DVE block (which stacked both DMA
                # windows at the period end).
                nc.vector.scalar_tensor_tensor(
                    out=sqx[b][0:1, 0:1], in0=other_pidx[0:1, 0:1], scalar=0.0,
                    in1=sqx[b][0:1, 0:1], op0=Alu.mult, op1=Alu.add)

            def body_iter(k_aps, q):
                for b in range(BPC):
                    phase0_xy(b)
                    phase0_z(b)
                fence(0, pidx[1][1 - q])  # b1's maxidx of previous iteration
                phase1(0, q)
                phase2a(0, q)
                fence(1, pidx[0][q])      # b0's maxidx of this iteration
                phase1(1, q)
                phase2a(1, q)
                for b in range(BPC):
                    phase2b(b)
                for b in range(BPC):
                    nc.vector.tensor_scalar(out=cent[b][0:1, k_aps[b]],
                                            in0=wmax[b][0:1, 0:1], scalar1=-1.0,
                                            scalar2=C23, op0=Alu.mult, op1=Alu.add)

            n_iters = NPOINT - 1
            n_loop = (n_iters // UNROLL) * UNROLL
            if n_loop > 0:
                with tc.For_i(1, 1 + n_loop, UNROLL) as i:
                    for u in range(UNROLL):
                        body_iter([bass.ds(i + u, 1)] * BPC, u % 2)
            for k in range(1 + n_loop, NPOINT):
                body_iter([slice(k, k + 1)] * BPC, k % 2)

            # ---- final stage: s_obj, gather, normalize, emit ----
            from concourse.bass_isa import ReduceOp as RO
            for b in range(BPC):
                planes3 = msb[b][:].rearrange("p (c w) -> p c w", c=3)
                colsum = wp.tile([P, 3], FP32, name="colsum", tag="colsum")
                nc.vector.reduce_sum(out=colsum[:], in_=planes3, axis=X)
                sums_bc = wp.tile([P, 3], FP32, name="sumsbc", tag="sumsbc")
                nc.gpsimd.partition_all_reduce(sums_bc[:], colsum[:], P, RO.add)
                negmean = wp.tile([P, 3], FP32, name="negmean", tag="negmean")
                nc.vector.tensor_scalar(out=negmean[:], in0=sums_bc[:],
                                        scalar1=-1.0 / N, scalar2=None, op0=Alu.mult)
                sq = [wp.tile([P, W], FP32, name=f"fsq{c}", tag=f"fsq{c}") for c in range(3)]
                for c in range(3):
                    nc.scalar.activation(
                        out=sq[c][:], in_=plane(b, c),
                        func=Act.Square, bias=negmean[:, c:c + 1], scale=1.0)
                s = wp.tile([P, W], FP32, name="fs", tag="fs")
                nc.vector.tensor_tensor(out=s[:], in0=sq[0][:], in1=sq[1][:], op=Alu.add)
                nc.vector.tensor_tensor(out=s[:], in0=s[:], in1=sq[2][:], op=Alu.add)
                mcol = wp.tile([P, 1], FP32, name="mcol", tag="mcol")
                nc.vector.reduce_max(out=mcol[:], in_=s[:], axis=X)
                mbc = wp.tile([P, 1], FP32, name="mbc", tag="mbc")
                nc.gpsimd.partition_all_reduce(mbc[:], mcol[:], P, RO.max)
                scol = wp.tile([P, 1], FP32, name="scol", tag="scol")
                nc.scalar.activation(out=scol[:], in_=mbc[:], func=Act.Sqrt)
                y0 = wp.tile([P, 1], FP32, name="y0", tag="y0")
                nc.vector.reciprocal(out=y0[:], in_=scol[:])
                t0 = wp.tile([P, 1], FP32, name="t0", tag="t0")
                nc.vector.tensor_tensor(out=t0[:], in0=mbc[:], in1=y0[:], op=Alu.mult)
                nc.vector.tensor_tensor(out=t0[:], in0=t0[:], in1=y0[:], op=Alu.mult)
                nc.vector.tensor_scalar(out=t0[:], in0=t0[:], scalar1=-0.5,
                                        scalar2=1.5, op0=Alu.mult, op1=Alu.add)
                y1 = wp.tile([P, 1], FP32, name="y1", tag="y1")
                nc.vector.tensor_tensor(out=y1[:], in0=y0[:], in1=t0[:], op=Alu.mult)

                centT = pp.tile([P, NG], FP32, name="centT", tag="centT")
                for j in range(NG):
                    nc.tensor.transpose(out=centT[:, j:j + 1],
                                        in_=cent[b][0:1, j * P:(j + 1) * P],
                                        identity=ident[0:1, 0:1])
                offsg = wp.tile([P, NG], I32, name="offsg", tag="offsg")
                nc.scalar.activation(out=offsg[:], in_=centT[:],
                                     func=Act.Identity, bias=boffP[b][:], scale=1.0)
                obuf = wp.tile([P, NG * 4], FP32, name="obuf", tag="obuf")
                for j in range(NG):
                    pcj = wp.tile([P, 3], FP32, name="pcj", tag="pcj")
                    nc.gpsimd.indirect_dma_start(
                        out=pcj[:], out_offset=None, in_=meshflat_in[:],
                        in_offset=bass.IndirectOffsetOnAxis(ap=offsg[:, j:j + 1], axis=0))
                    nc.gpsimd.indirect_dma_start(
                        out=obuf[:, 4 * j:4 * j + 1], out_offset=None, in_=cmflat_in[:],
                        in_offset=bass.IndirectOffsetOnAxis(ap=offsg[:, j:j + 1], axis=0))
                    nc.vector.tensor_scalar(out=obuf[:, 4 * j + 1:4 * j + 4],
                                            in0=pcj[:], scalar1=y1[:], scalar2=None,
                                            op0=Alu.mult)
                out_ap = out_t[b].rearrange("(j p) f -> p j f", p=P)
                nc.sync.dma_start(out=out_ap,
                                  in_=obuf[:].rearrange("p (j f) -> p j f", f=4))
                if debug:
                    nc.sync.dma_start(out=cent_out[b].rearrange("(o n) -> o n", o=1),
                                      in_=cent[b][:])

    nc.finalize()
    return nc, W


def _prep_core_inputs(mesh, contact, init, N, W, core):
    """Per-core input map for batches [2c, 2c+1]; planar mesh layout."""
    bsel = [BPC * core + i for i in range(BPC)]
    mesh_sb = np.zeros((BPC, P, 3 * W), np.float32)
    d0 = np.full((BPC, P * W), -1.0, np.float32)
    negc0 = np.zeros((BPC, P, 3), np.float32)
    centinit = np.zeros((1, BPC), np.float32)
    for i, b in enumerate(bsel):
        buf = np.zeros((P * W, 3), np.float32)
        buf[:N] = mesh[b]
        # [P, W, 3] -> [P, 3, W] planar
        mesh_sb[i] = buf.reshape(P, W, 3).transpose(0, 2, 1).reshape(P, 3 * W)
        d0[i, :N] = 1e10
        init_idx = int(init[b])
        negc0[i, :, :] = -mesh[b, init_idx][None, :]
        centinit[0, i] = float(init_idx)
    d0 = d0.reshape(BPC, P, W)
    PW = P * W
    meshflat = np.zeros((BPC * PW, 3), np.float32)
    cmflat = np.zeros((BPC * PW, 1), np.float32)
    for i, b in enumerate(bsel):
        meshflat[i * PW:i * PW + N] = mesh[b]
        cmflat[i * PW:i * PW + N] = contact[b]
    pwfm = (C23 - np.arange(P, dtype=np.float32) * W).reshape(P, 1)
    pwf = (np.arange(P, dtype=np.float32) * W).reshape(P, 1)
    pwfbn = np.stack([(np.arange(P, dtype=np.float32) * W + b * PW).reshape(P, 1)
                      for b in range(BPC)])
    return {
        "mesh_sb": mesh_sb, "meshflat": meshflat, "cmflat": cmflat,
        "d0": d0, "negc0": negc0, "pwfm": pwfm, "pwf": pwf, "pwfbn": pwfbn,
        "centinit": centinit,
    }


def _get_built(N=N_FULL, NPOINT=NPOINT_FULL, UNROLL=31, WZA=261, ADD_DVE=250,
               debug=False):
    key = (N, NPOINT, UNROLL, WZA, ADD_DVE, debug)
    if key not in _BUILD_CACHE:
        _BUILD_CACHE[key] = _build(N, NPOINT, UNROLL, WZA, ADD_DVE, debug)
    return _BUILD_CACHE[key]


def kernel(mesh, contact_map, init_farthest, npoint):
    from concourse.bass_utils import run_bass_kernel_spmd

    mesh = np.asarray(mesh, np.float32)
    contact_map = np.asarray(contact_map, np.float32)
    init_farthest = np.asarray(init_farthest, np.int32)
    assert int(npoint) == NPOINT_FULL and mesh.shape == (B_FULL, N_FULL, 3)

    nc, W = _get_built()
    in_maps = [
        _prep_core_inputs(mesh, contact_map, init_farthest, N_FULL, W, c)
        for c in range(NCORES)
    ]
    res = run_bass_kernel_spmd(nc, in_maps, list(range(NCORES)))
    out = np.concatenate([res.results[c]["out"] for c in range(NCORES)], axis=0)
    return out.astype(np.float32)
